# revision 1
# baseline (speedup 1.0000x reference)
"""Trainium2 Bass kernel for DescriptorMatchLoss (retrieval_knn).

Reference computation (per batch-pair grid [B,B]):
    d2[i,j,n,m] = ||denorm(pts_src[i,n]) - denorm(pts_dst[i,j,m])||^2
    mask        = d2 <= RADIUS^2
    cos[i,j,n,m] = <fhat[j,n], fhat[i,m]>   (fhat = row-normalized features)
    loss = sum(mask * (1 - cos)) / max(sum(mask), 1)

Device strategy (8 cores, 2 (i,j) pairs per core):
  * z = 64 - d2 tile [128n, mw] via one K=14 bf16 matmul: coordinates are
    split into (hi, lo) bf16 pairs so every product is exact in fp32 PSUM
    (full PE rate; native fp32 matmul is 4x slower).
  * Mask tiles in SBUF bf16, produced alternately by the ACT engine
    (sign(z) in {-1,0,+1}, fused count accumulation) and the DVE
    (z >= 0 in {1,0}) so PSUM slots recycle fast enough to keep PE fed.
  * PE contracts G[m,d] = sum_n mask[n,m] * fhat[j][n,d] (mask stationary,
    K=128 per n-tile, accumulated over 16 n-tiles in PSUM).
  * DVE multiply+reduce: ext = sum_{m,d} G[m,d]*fhat[i][m,d]
    = sum_{n,m} mask[n,m]*cos[n,m].
  * Host: exact affine correction for the +-1 tiles (sum of cos over a
    full n-range x m-chunk factorizes into dots of feature column sums).

kernel(**inputs) takes FULL inputs, shards pairs across 8 cores, returns the
scalar loss (fp32).
"""

import sys

for _p in ("/opt/pypackages", "/opt/trn_rl_repo"):
    if _p not in sys.path:
        sys.path.insert(0, _p)

import numpy as np
import ml_dtypes

BF16 = ml_dtypes.bfloat16

# Problem constants (hardcoded per contract).
B, N, D = 4, 2048, 256
HEIGHT, WIDTH = 480, 640
RADIUS2 = 64.0
N_CORES = 8
PAIRS_PER_CORE = (B * B) // N_CORES  # 2

P = 128          # partitions
NT = N // P      # 16 n-tiles of 128
DC = D // P      # feature-dim chunks (2)
KGEO = 14        # geometry contraction rows

# Tunables (kernel structure); _host_prep must agree on MW/engine split.
MW = 512         # m-tile width
MT = N // MW     # m-tiles per pair
MC = MW // P     # m-chunks of 128 per m-tile
D2_BUFS = 6
G_BUFS = 2
MASK_BUFS = 7
PIPE = True      # software-pipeline G one step behind d2/sign
REPS = 1         # repeat compute loop (timing only)
USE_TTR = False  # fused multiply+reduce extraction (walrus rejects)
CARRIER = False  # tiny PE matmul absorbing the g-slot WAR wait (the hoisted
                 # eventsem from _split_multi_waits is cheaper on HW)
EXT_PATH = "dve"  # "dve": DVE TT+reduce from PSUM; "pool": ACT copy ->
                  # GpSimd multiply -> DVE reduce (spreads extraction load)
FJ_FP8 = True    # fp8e4m3 fj + masks, G matmul in DoubleRow mode (2x fewer
                 # PE contraction steps; loss impact ~2e-6 rel, host-corrected
                 # exactly via fp8 column sums)


# Per-step engine pattern for mask production, chosen to balance engine
# load (DVE also runs the extraction): 5 ACT steps, 3 DVE steps, with the
# pipeline's step-pairs mixed (ACT, DVE) where possible.
MASK_PATTERN = ["act", "dve", "act", "dve", "act", "dve", "act", "act"]


def _mask_engine(pair, mt, nt=None):
    """Which engine produces the mask for (pair, mt): 'act' (+-1 sign
    convention) or 'dve' ({0,1} convention); uniform per step so the
    host-side affine correction stays exact."""
    return MASK_PATTERN[(pair * MT + mt) % len(MASK_PATTERN)]


_CACHE = {}
LAST = None  # BassKernelResults of the most recent run (for test harness)


def _build_bass(reps=None, mode="full", split_waits=True):
    import concourse.bass as bass
    import concourse.mybir as mybir
    import concourse.tile as tile

    if reps is None:
        reps = REPS

    nc = bass.Bass(trn_type="TRN2", target_bir_lowering=False, debug=False)
    f32 = mybir.dt.float32
    bf16 = mybir.dt.bfloat16

    mdt = mybir.dt.float8e4 if FJ_FP8 else bf16  # fj + mask dtype
    fj_d = nc.dram_tensor("fj", [PAIRS_PER_CORE, N, D], mdt, kind="ExternalInput")
    # fiT: host-transposed normalized features of the i-batches, [pairs, D, N]
    fi_d = nc.dram_tensor("fiT", [PAIRS_PER_CORE, D, N], bf16, kind="ExternalInput")
    geoL_d = nc.dram_tensor(
        "geoL", [PAIRS_PER_CORE, KGEO, N], bf16, kind="ExternalInput"
    )
    geoR_d = nc.dram_tensor(
        "geoR", [PAIRS_PER_CORE, KGEO, N], bf16, kind="ExternalInput"
    )
    # out[:, 0:PAIRS*MT*NT]      : per-(pair, m-tile, n-tile) mask sums
    # out[:, PAIRS*MT*NT:+32]    : per-(pair, m-chunk) mask*cos sums
    n_sgn = PAIRS_PER_CORE * MT * NT
    n_ext = PAIRS_PER_CORE * MT * DC
    out_d = nc.dram_tensor("out", [P, n_sgn + n_ext], f32, kind="ExternalOutput")

    steps = [(pair, mt) for pair in range(PAIRS_PER_CORE) for mt in range(MT)]

    with tile.TileContext(nc) as tc:
        with (
            tc.tile_pool(name="feat", bufs=1) as feat_pool,
            tc.tile_pool(name="geo", bufs=1) as geo_pool,
            tc.tile_pool(name="acc", bufs=1) as acc_pool,
            tc.tile_pool(name="mask", bufs=MASK_BUFS) as mask_pool,
            tc.tile_pool(name="scratch", bufs=2) as scratch_pool,
            tc.tile_pool(name="psum_d2", bufs=D2_BUFS, space="PSUM") as d2_pool,
            tc.tile_pool(name="psum_g", bufs=G_BUFS, space="PSUM") as g_pool,
        ):
            fj_sb = feat_pool.tile([P, PAIRS_PER_CORE, NT, D], mdt)
            fi_sb = feat_pool.tile([P, PAIRS_PER_CORE, DC, N], bf16)
            # Geometry replicated at partition offsets 0/32/64/96 so four
            # K=14 d2 matmuls can run concurrently in the four PE row groups.
            geoL_sb = geo_pool.tile([P, PAIRS_PER_CORE, N], bf16)
            geoR_sb = geo_pool.tile([P, PAIRS_PER_CORE, N], bf16)
            sgn_acc = acc_pool.tile([P, n_sgn], f32)
            ext_acc = acc_pool.tile([P, n_ext], f32)
            if mode != "full":
                nc.vector.memset(sgn_acc[:], 0.0)
                nc.vector.memset(ext_acc[:], 0.0)

            for rg in range(4):
                nc.sync.dma_start(
                    out=geoL_sb[32 * rg : 32 * rg + KGEO, :, :],
                    in_=geoL_d[:].rearrange("q k n -> k q n"),
                )
                nc.sync.dma_start(
                    out=geoR_sb[32 * rg : 32 * rg + KGEO, :, :],
                    in_=geoR_d[:].rearrange("q k n -> k q n"),
                )
            nc.sync.dma_start(
                out=fj_sb[:], in_=fj_d[:].rearrange("q (t p) d -> p q t d", p=P)
            )
            nc.sync.dma_start(
                out=fi_sb[:], in_=fi_d[:].rearrange("q (c p) n -> p q c n", p=P)
            )

            # DMA-tick absorbers: each engine "observes" the input-DMA
            # completion once via a cheap op, so later instructions inherit
            # the tick through the vector clock and mostly carry a single
            # cross-engine wait.
            dummy_ps = g_pool.tile([1, 8], f32, tag="g")
            dummy_sb = scratch_pool.tile([1, 8], f32, tag="dmy")
            nc.tensor.matmul(
                dummy_ps[:, 0:8], geoL_sb[0:KGEO, 0, 0:1], geoL_sb[0:KGEO, 0, 0:8],
                start=True, stop=True,
            )
            nc.tensor.matmul(
                dummy_ps[:, 0:8], geoR_sb[0:KGEO, 0, 0:1], geoR_sb[0:KGEO, 0, 0:8],
                start=True, stop=True,
            )
            nc.tensor.matmul(
                dummy_ps[:, 0:8], fj_sb[:, 0, 0, 0:1], fj_sb[:, 0, 0, 0:8],
                start=True, stop=True,
            )
            nc.vector.tensor_copy(dummy_sb[0:1, 0:1], fi_sb[0:1, 0, 0, 0:1])
            nc.scalar.copy(dummy_sb[0:1, 1:2], dummy_sb[0:1, 0:1])

            def emit_mask_op(pair, mt, nt, d2_ps, mask_t):
                col = (pair * MT + mt) * NT + nt
                eng = _mask_engine(pair, mt, nt)
                if eng == "act":
                    nc.scalar.activation(
                        mask_t[:, nt, :],
                        d2_ps[:],
                        mybir.ActivationFunctionType.Sign,
                        accum_out=sgn_acc[:, col : col + 1],
                    )
                else:
                    nc.vector.tensor_scalar(
                        out=mask_t[:, nt, :],
                        in0=d2_ps[:],
                        scalar1=0.0,
                        scalar2=0.0,
                        op0=mybir.AluOpType.is_ge,
                        op1=mybir.AluOpType.add,
                        accum_out=sgn_acc[:, col : col + 1],
                    )

            def emit_d2_quad(pair, mt, nt0, mask_t):
                """Four K=14 d2 matmuls packed into the four PE row groups
                (concurrent execution; weight loads overlap other groups'
                in-flight matmuls), then their mask ops."""
                tiles = []
                for k in range(4):
                    nt = nt0 + k
                    d2_ps = d2_pool.tile([P, MW], f32, tag="d2")
                    nc.tensor.matmul(
                        d2_ps[:],
                        geoL_sb[32 * k : 32 * k + KGEO, pair,
                                nt * P : (nt + 1) * P],
                        geoR_sb[32 * k : 32 * k + KGEO, pair,
                                mt * MW : (mt + 1) * MW],
                        start=True,
                        stop=True,
                        tile_position=(32 * k, 0),
                    )
                    tiles.append((nt, d2_ps))
                for nt, d2_ps in tiles:
                    emit_mask_op(pair, mt, nt, d2_ps, mask_t)

            def emit_d2_sign(pair, mt):
                mask_t = mask_pool.tile([P, NT, MW], mdt, tag="mask")
                for q in range(NT // 4):
                    emit_d2_quad(pair, mt, 4 * q, mask_t)
                return mask_t

            def emit_d2_sign_pair(sa, sb):
                """Interleave two steps' d2+mask production so the ACT-masked
                and DVE-masked streams run concurrently."""
                ma = mask_pool.tile([P, NT, MW], mdt, tag="mask")
                mb = mask_pool.tile([P, NT, MW], mdt, tag="mask")
                for q in range(NT // 4):
                    emit_d2_quad(sa[0], sa[1], 4 * q, ma)
                    emit_d2_quad(sb[0], sb[1], 4 * q, mb)
                return ma, mb

            def emit_g_half(pair, mt, mask_t, dc, half, g_ps):
                # G^T[d, m] = sum_n fhat_j[n, d] * mask[n, m]: stationary is
                # the fhat_j (n x d-chunk) tile, moving is the full [128, MW]
                # mask tile, so each matmul streams MW columns and the weight
                # load hides behind the previous matmul.
                if half == 0 and CARRIER:
                    # Carrier: absorb the WAR wait on this PSUM slot (its
                    # previous reader was the DVE extraction) into a tiny
                    # matmul so the real group's first matmul only waits
                    # on the mask writes.
                    nc.tensor.matmul(
                        g_ps[0:1, 0:1],
                        geoL_sb[0:KGEO, pair, 0:1],
                        geoR_sb[0:KGEO, pair, 0:1],
                        start=True,
                        stop=True,
                    )
                if FJ_FP8:
                    # DoubleRow: each matmul contracts TWO n-tiles (fp8 pairs
                    # interleaved along the middle AP dim).
                    nps = NT // 2  # 8 nt-pairs
                    prs = range(nps // 2) if half == 0 else range(nps // 2, nps)
                    for t in prs:
                        nc.tensor.matmul(
                            g_ps[:],
                            fj_sb[:, pair, 2 * t : 2 * t + 2,
                                  dc * P : (dc + 1) * P],
                            mask_t[:, 2 * t : 2 * t + 2, :],
                            start=(t == 0),
                            stop=(t == nps - 1),
                            perf_mode=mybir.MatmulPerfMode.DoubleRow,
                        )
                else:
                    nts = range(NT // 2) if half == 0 else range(NT // 2, NT)
                    for nt in nts:
                        nc.tensor.matmul(
                            g_ps[:],
                            fj_sb[:, pair, nt, dc * P : (dc + 1) * P],
                            mask_t[:, nt, :],
                            start=(nt == 0),
                            stop=(nt == NT - 1),
                        )
                if half == 1:
                    ecol = (pair * MT + mt) * DC + dc
                    scr = scratch_pool.tile([P, MW], f32, tag="scr")
                    if USE_TTR:
                        nc.vector.tensor_tensor_reduce(
                            out=scr[:],
                            in0=g_ps[:],
                            in1=fi_sb[:, pair, dc, mt * MW : (mt + 1) * MW],
                            scale=1.0,
                            scalar=0.0,
                            op0=mybir.AluOpType.mult,
                            op1=mybir.AluOpType.add,
                            accum_out=ext_acc[:, ecol : ecol + 1],
                        )
                    elif EXT_PATH == "pool":
                        g_sb = scratch_pool.tile([P, MW], f32, tag="gsb")
                        nc.scalar.copy(g_sb[:], g_ps[:])
                        nc.gpsimd.tensor_tensor(
                            out=scr[:],
                            in0=g_sb[:],
                            in1=fi_sb[:, pair, dc, mt * MW : (mt + 1) * MW],
                            op=mybir.AluOpType.mult,
                        )
                        nc.vector.tensor_reduce(
                            out=ext_acc[:, ecol : ecol + 1],
                            in_=scr[:],
                            axis=mybir.AxisListType.X,
                            op=mybir.AluOpType.add,
                        )
                    else:
                        nc.vector.tensor_tensor(
                            out=scr[:],
                            in0=g_ps[:],
                            in1=fi_sb[:, pair, dc, mt * MW : (mt + 1) * MW],
                            op=mybir.AluOpType.mult,
                        )
                        nc.vector.tensor_reduce(
                            out=ext_acc[:, ecol : ecol + 1],
                            in_=scr[:],
                            axis=mybir.AxisListType.X,
                            op=mybir.AluOpType.add,
                        )

            def g_units(pair, mt, mask_t):
                units = []
                for dc in range(DC):
                    g_ps = g_pool.tile([P, MW], f32, tag="g")
                    for half in range(2):
                        units.append(
                            (lambda p=pair, m=mt, k=mask_t, d=dc, h=half,
                             g=g_ps: emit_g_half(p, m, k, d, h, g))
                        )
                return units

            def emit_g(pair, mt, mask_t):
                for u in g_units(pair, mt, mask_t):
                    u()

            def emit_body(mode):
                if mode == "d2sign":
                    for s in steps:
                        emit_d2_sign(*s)
                elif mode == "d2only":
                    for pair, mt in steps:
                        for q in range(NT // 4):
                            for k in range(4):
                                nt = 4 * q + k
                                d2_ps = d2_pool.tile([P, MW], f32, tag="d2")
                                nc.tensor.matmul(
                                    d2_ps[:],
                                    geoL_sb[32 * k : 32 * k + KGEO, pair,
                                            nt * P : (nt + 1) * P],
                                    geoR_sb[32 * k : 32 * k + KGEO, pair,
                                            mt * MW : (mt + 1) * MW],
                                    start=True,
                                    stop=True,
                                    tile_position=(32 * k, 0),
                                )
                elif mode == "gonly":
                    mask_const = mask_pool.tile([P, NT, MW], mdt, tag="mask")
                    nc.vector.memset(mask_const[:], 1.0)
                    for pair, mt in steps:
                        emit_g(pair, mt, mask_const)
                elif PIPE:
                    # Software pipeline: phase k's d2-quads+masks interleave
                    # with phase k-1's G units so the PE's in-order queue
                    # always has ready G work while masks drain d2 slots.
                    prev_units = None
                    for k in range(0, len(steps), 2):
                        sa, sb = steps[k], steps[k + 1]
                        ma = mask_pool.tile([P, NT, MW], mdt, tag="mask")
                        mb = mask_pool.tile([P, NT, MW], mdt, tag="mask")
                        quads = []
                        for q in range(NT // 4):
                            quads.append(
                                lambda s=sa, m=ma, q0=4 * q:
                                    emit_d2_quad(s[0], s[1], q0, m)
                            )
                            quads.append(
                                lambda s=sb, m=mb, q0=4 * q:
                                    emit_d2_quad(s[0], s[1], q0, m)
                            )
                        for idx, qu in enumerate(quads):
                            # Quad first: the d2 quads feed the mask engines
                            # as early as possible (measured better than
                            # G-unit-first, which starves mask production).
                            qu()
                            if prev_units is not None:
                                prev_units[idx]()
                        prev_units = (
                            g_units(sa[0], sa[1], ma) + g_units(sb[0], sb[1], mb)
                        )
                    for u in prev_units:
                        u()
                else:
                    for s in steps:
                        m = emit_d2_sign(*s)
                        emit_g(s[0], s[1], m)

            if reps == 1:
                emit_body(mode)
            else:
                with tc.For_i(0, reps, 1):
                    emit_body(mode)

            nc.sync.dma_start(out=out_d[:, 0:n_sgn], in_=sgn_acc[:])
            nc.sync.dma_start(out=out_d[:, n_sgn : n_sgn + n_ext], in_=ext_acc[:])

    if split_waits:
        _split_multi_waits(nc)
    return nc


def _split_multi_waits(nc):
    """Walrus rejects >1 sync-wait on compute/DMA instruction encodings.

    Hoist all but one wait of any multi-wait instruction onto standalone
    InstEventSemaphore instructions inserted immediately before it on the
    same engine queue (semantically identical: every wait must pass before
    the instruction dispatches either way).
    """
    import concourse.mybir as mybir

    n_split = 0
    for bb in nc.main_func.blocks:
        new_list = []
        for inst in bb.instructions:
            si = inst.sync_info
            if (
                si is not None
                and si.on_wait
                and len(si.on_wait) > 1
                and not isinstance(inst, mybir.InstEventSemaphore)
            ):
                waits = list(si.on_wait)
                for k, w in enumerate(waits[:-1]):
                    n_split += 1
                    new_list.append(
                        mybir.InstEventSemaphore(
                            name=f"{inst.name}-hw{k}",
                            engine=inst.engine,
                            ins=[],
                            outs=[],
                            sync_info=mybir.SyncInfo(on_wait=[w], on_update=[]),
                        )
                    )
                inst.sync_info = mybir.SyncInfo(
                    on_wait=[waits[-1]], on_update=list(si.on_update or [])
                )
            new_list.append(inst)
        bb.instructions[:] = new_list
    return n_split


def _get_bass():
    if "nc" not in _CACHE:
        _CACHE["nc"] = _build_bass()
    return _CACHE["nc"]


def _split2(x):
    """fp64 -> (hi, lo) bf16 such that hi+lo ~ x to ~17 mantissa bits."""
    hi = x.astype(BF16)
    lo = (x - hi.astype(np.float64)).astype(BF16)
    return hi, lo


def _split3(x):
    hi = x.astype(BF16)
    r = x - hi.astype(np.float64)
    mid = r.astype(BF16)
    lo = (r - mid.astype(np.float64)).astype(BF16)
    return hi, mid, lo


def _host_prep(features, pts_src, pts_dst, height, width):
    """Build per-core device inputs + exact host-side correction terms."""
    height = int(height)
    width = int(width)
    scale32 = np.array(
        [(width - 1) * 0.5, (height - 1) * 0.5], dtype=np.float32
    )

    # Match the reference's fp32 denormalization rounding, then center (the
    # centering offset equals `scale`, so centered coords = denorm - scale).
    ps32 = (pts_src.astype(np.float32) + np.float32(1.0)) * scale32  # [B,N,2]
    pd32 = (pts_dst.astype(np.float32) + np.float32(1.0)) * scale32  # [B,B,N,2]
    psc = ps32.astype(np.float64) - scale32.astype(np.float64)
    pdc = pd32.astype(np.float64) - scale32.astype(np.float64)

    phx, plx = _split2(psc[..., 0])
    phy, ply = _split2(psc[..., 1])
    qhx, qlx = _split2(pdc[..., 0])
    qhy, qly = _split2(pdc[..., 1])

    # The PSUM result is z = 64 - d2 = 2 p.q + (64 - s_src) - s_dst, so the
    # mask is sign(z) / (z >= 0) with no activation bias needed.  s terms are
    # computed from the *split* values so the only error is the residual.
    sh, sm, sl = _split3(
        RADIUS2
        - (
            (phx.astype(np.float64) + plx.astype(np.float64)) ** 2
            + (phy.astype(np.float64) + ply.astype(np.float64)) ** 2
        )
    )  # [B,N]
    tq = (
        (qhx.astype(np.float64) + qlx.astype(np.float64)) ** 2
        + (qhy.astype(np.float64) + qly.astype(np.float64)) ** 2
    )
    th, tm, tl = _split3(tq)  # [B,B,N]

    ones_bn = np.ones((B, N), dtype=BF16)
    ones_bbn = np.ones((B, B, N), dtype=BF16)
    neg_ones_bn = -ones_bn

    p2hx = (2.0 * phx.astype(np.float64)).astype(BF16)
    p2lx = (2.0 * plx.astype(np.float64)).astype(BF16)
    p2hy = (2.0 * phy.astype(np.float64)).astype(BF16)
    p2ly = (2.0 * ply.astype(np.float64)).astype(BF16)
    geoL_all = np.stack(
        [p2hx, p2hx, p2lx, p2lx, p2hy, p2hy, p2ly, p2ly,
         sh, sm, sl, neg_ones_bn, neg_ones_bn, neg_ones_bn],
        axis=1,
    )  # [B, 14, N]
    geoR_all = np.stack(
        [qhx, qlx, qhx, qlx, qhy, qly, qhy, qly,
         ones_bbn, ones_bbn, ones_bbn, th, tm, tl],
        axis=2,
    )  # [B, B, 14, N]

    # Normalized features, rounded to bf16 (the dtype used on device).
    f64 = features.astype(np.float64)
    norms = np.sqrt((f64 * f64).sum(-1, keepdims=True))
    fhat = (f64 / norms).astype(BF16)  # [B, N, D]
    if FJ_FP8:
        fhat_j = fhat.astype(ml_dtypes.float8_e4m3)  # device fj operand
    else:
        fhat_j = fhat

    # Per-m-chunk column sums for the +-1 correction (exact, fp64 over the
    # same quantized values the device uses: fj-side dtype for `fsum`,
    # bf16 fiT for `fsum_chunk`).
    fsum_chunk = fhat.astype(np.float64).reshape(B, NT, P, D).sum(axis=2)
    fsum = fhat_j.astype(np.float64).sum(axis=1)  # [B, D]

    in_maps = []
    pair_idx = []  # per core: list of (i, j)
    for c in range(N_CORES):
        pairs = [2 * c, 2 * c + 1]
        ii = [p // B for p in pairs]
        jj = [p % B for p in pairs]
        in_maps.append(
            {
                "fj": np.ascontiguousarray(fhat_j[jj]),
                "fiT": np.ascontiguousarray(fhat[ii].transpose(0, 2, 1)),
                "geoL": np.ascontiguousarray(geoL_all[ii]),
                "geoR": np.ascontiguousarray(
                    np.stack([geoR_all[i_, j_] for i_, j_ in zip(ii, jj)])
                ),
            }
        )
        pair_idx.append(list(zip(ii, jj)))
    return in_maps, pair_idx, fsum, fsum_chunk


def _combine(results, pair_idx, fsum, fsum_chunk, cores=None):
    """Host-side exact combination of per-core partial sums."""
    if cores is None:
        cores = range(len(results))
    n_sgn = PAIRS_PER_CORE * MT * NT
    a_total = 0.0
    b_total = 0.0
    for c in cores:
        out = results[c]["out"].astype(np.float64)
        sgn_p = out[:, 0:n_sgn]                    # per-partition accum values
        ext = out[:, n_sgn:].sum(axis=0)           # per (pair, mt, dc) col
        for p, (i_, j_) in enumerate(pair_idx[c]):
            for mt in range(MT):
                for nt in range(NT):
                    eng = _mask_engine(p, mt, nt)
                    col = sgn_p[:, (p * MT + mt) * NT + nt]
                    if eng == "act":
                        # sum of +-1 per partition over MW elements
                        a_total += 0.5 * (float(col.sum()) + P * MW)
                    else:
                        a_total += float(col.sum())  # {0,1} masks
            for mt in range(MT):
                eng0 = _mask_engine(p, mt, 0)
                # m-tile column sums of fhat_i over this tile's m range
                fs_mt = fsum_chunk[i_, mt * MC : (mt + 1) * MC].sum(axis=0)
                for dc in range(DC):
                    e = float(ext[(p * MT + mt) * DC + dc])
                    if eng0 == "act":
                        # +-1 convention
                        dsl = slice(dc * P, (dc + 1) * P)
                        corr = float(np.dot(fsum[j_][dsl], fs_mt[dsl]))
                        b_total += 0.5 * (e + corr)
                    else:
                        b_total += e
    return a_total, b_total


def kernel(features, pts_src, pts_dst, invis_idx, height, width):
    global LAST
    del invis_idx  # unused by the reference computation

    features = np.asarray(features)
    pts_src = np.asarray(pts_src)
    pts_dst = np.asarray(pts_dst)

    in_maps, pair_idx, fsum, fsum_chunk = _host_prep(
        features, pts_src, pts_dst, height, width
    )

    from concourse.bass_utils import run_bass_kernel_spmd

    nc = _get_bass()
    LAST = run_bass_kernel_spmd(nc, in_maps, core_ids=list(range(N_CORES)))

    a_total, b_total = _combine(LAST.results, pair_idx, fsum, fsum_chunk)
    loss = (a_total - b_total) / max(a_total, 1.0)
    return np.float32(loss)



# revision 47
# speedup vs baseline: 3.8049x; 3.8049x over previous
"""Trainium2 Bass kernel for DescriptorMatchLoss (retrieval_knn).

Reference computation (per batch-pair grid [B,B]):
    d2[i,j,n,m] = ||denorm(pts_src[i,n]) - denorm(pts_dst[i,j,m])||^2
    mask        = d2 <= RADIUS^2
    cos[i,j,n,m] = <fhat[j,n], fhat[i,m]>   (fhat = row-normalized features)
    loss = sum(mask * (1 - cos)) / max(sum(mask), 1)

Block-sparse device strategy (8 cores, 2 (i,j) pairs per core):
  * Rows (n) of each pair are sorted by the x coordinate of the source
    point, columns (m) by the x coordinate of the destination point
    (host-side permutations).  A mask block can then only be nonzero when
    the 128-row tile's x-range and the 256-column chunk's x-range are
    within RADIUS of each other — for uniform points that keeps only
    ~23% of the [N,N] grid (contiguous n-tile "runs" per column chunk).
    Skipped blocks are *exactly* zero, so the result is unchanged.
  * Runs are unioned across the 8 cores so one SPMD program serves all.
  * Per active unit u = (slot, mc): a [128, L*256] z-tile (z = R^2 - d2)
    via L K=14 bf16 matmuls (hi/lo-split coordinates, exact in fp32),
    then ONE mask op covering the whole run:
      - ACT units: Sign(z) in {-1,0,1} fp8, accum_out = sum (count via
        affine fix on host)
      - DVE units: z >= 0 in {1,0} fp8, accum_out = count
  * G''[m,d] = sum_n mask[n,m] * fj8[n,d] with the mask as the matmul
    stationary operand (fp8 DoubleRow over n-tile pairs) and fj8 moving,
    accumulated over the run in a [128, 2*256] PSUM tile (m-chunk halves
    side by side).
  * Extraction: ONE fused scalar_tensor_tensor per unit:
      accum_out = sum_d,m G''[m,d] * fi8[m,d]   (internal fp32 reduce)
  * Host combines: per-ACT-unit affine corrections (exact fp64 from the
    fp8-quantized operands), then loss = (A - B) / max(A, 1).

kernel(**inputs) takes FULL inputs, shards pairs across 8 cores, returns
the scalar loss (fp32).  The bass program is specialized on the sparsity
structure of the actual inputs (recompiled if the geometry changes).
"""

import sys

for _p in ("/opt/pypackages", "/opt/trn_rl_repo"):
    if _p not in sys.path:
        sys.path.insert(0, _p)

import numpy as np
import ml_dtypes

BF16 = ml_dtypes.bfloat16
FP8 = ml_dtypes.float8_e4m3

# Problem constants (hardcoded per contract).
B, N, D = 4, 2048, 256
HEIGHT, WIDTH = 480, 640
RADIUS = 8.0
RADIUS2 = RADIUS * RADIUS
RUN_MARGIN = 0.02          # px slack when deciding block runs
N_CORES = 8
SLOTS = 2                  # (i,j) pairs per core
P = 128                    # partitions
NT = N // P                # 16 n-tiles
MC = 256                   # m-chunk width (mask/d2 block width)
NMC = N // MC              # 8 column chunks per pair
NU = SLOTS * NMC           # 16 units per core
KGEO = 14                  # geometry contraction rows
MAX_L = 4                  # d2 run-tile capacity (n-tiles per PSUM tile)

SLOT_MAJOR = False   # emission order groups slot 0 before slot 1
TAIL_DVE = True      # last emitted unit uses the DVE mask engine
OUT_CHUNK = 16       # units per chunked result DMA

_CACHE = {}
LAST = None  # BassKernelResults of the most recent run (for test harness)


# ---------------------------------------------------------------- host prep

def _split2(x):
    hi = x.astype(BF16)
    lo = (x - hi.astype(np.float64)).astype(BF16)
    return hi, lo


def _split3(x):
    hi = x.astype(BF16)
    r = x - hi.astype(np.float64)
    mid = r.astype(BF16)
    lo = (r - mid.astype(np.float64)).astype(BF16)
    return hi, mid, lo


def _geo_rows(psc, pdc):
    """14-row geometry operands such that
    z[n, m] = RADIUS2 - ||p_n - q_m||^2 = sum_k L[k, n] * R[k, m],
    exact to ~fp32 via bf16 hi/lo splits.  psc: [N,2] fp64 centered
    source coords, pdc: [N,2] fp64 centered dest coords."""
    phx, plx = _split2(psc[:, 0])
    phy, ply = _split2(psc[:, 1])
    qhx, qlx = _split2(pdc[:, 0])
    qhy, qly = _split2(pdc[:, 1])

    sh, sm, sl = _split3(
        RADIUS2
        - (
            (phx.astype(np.float64) + plx.astype(np.float64)) ** 2
            + (phy.astype(np.float64) + ply.astype(np.float64)) ** 2
        )
    )
    tq = (
        (qhx.astype(np.float64) + qlx.astype(np.float64)) ** 2
        + (qhy.astype(np.float64) + qly.astype(np.float64)) ** 2
    )
    th, tm, tl = _split3(tq)

    ones = np.ones((N,), dtype=BF16)
    p2hx = (2.0 * phx.astype(np.float64)).astype(BF16)
    p2lx = (2.0 * plx.astype(np.float64)).astype(BF16)
    p2hy = (2.0 * phy.astype(np.float64)).astype(BF16)
    p2ly = (2.0 * ply.astype(np.float64)).astype(BF16)
    geoL = np.stack(
        [p2hx, p2hx, p2lx, p2lx, p2hy, p2hy, p2ly, p2ly,
         sh, sm, sl, -ones, -ones, -ones], axis=0)          # [14, N]
    geoR = np.stack(
        [qhx, qlx, qhx, qlx, qhy, qly, qhy, qly,
         ones, ones, ones, th, tm, tl], axis=0)             # [14, N]
    return geoL, geoR


def _host_prep(features, pts_src, pts_dst, height, width):
    height = int(height)
    width = int(width)
    scale32 = np.array([(width - 1) * 0.5, (height - 1) * 0.5],
                       dtype=np.float32)

    # Match the reference's fp32 denorm rounding, then center.
    ps32 = (pts_src.astype(np.float32) + np.float32(1.0)) * scale32
    pd32 = (pts_dst.astype(np.float32) + np.float32(1.0)) * scale32
    psc = ps32.astype(np.float64) - scale32.astype(np.float64)  # [B,N,2]
    pdc = pd32.astype(np.float64) - scale32.astype(np.float64)  # [B,B,N,2]

    f64 = features.astype(np.float64)
    norms = np.sqrt((f64 * f64).sum(-1, keepdims=True))
    fhat8 = (f64 / norms).astype(BF16).astype(FP8)              # [B,N,D]

    # Per-core pair assignment: core c -> i = c//2, j_s = (2c+s) % B.
    cores = []
    for c in range(N_CORES):
        i = (2 * c) // B
        js = [(2 * c + s) % B for s in range(SLOTS)]
        rho = np.argsort(psc[i, :, 0], kind="stable")           # row perm
        sigs = [np.argsort(pdc[i, j, :, 0], kind="stable") for j in js]
        cores.append((i, js, rho, sigs))

    # Block runs per (slot, mc), unioned across cores so the single SPMD
    # program covers every core's sparsity pattern exactly.
    runs = np.zeros((SLOTS, NMC, 2), dtype=np.int64)  # (lo, hi)
    runs[:, :, 0] = NT
    runs[:, :, 1] = 0
    for c in range(N_CORES):
        i, js, rho, sigs = cores[c]
        rsx = psc[i, rho, 0]
        rlo = rsx[0::P]
        rhi = rsx[P - 1::P]
        for s in range(SLOTS):
            csx = pdc[i, js[s], sigs[s], 0]
            for mc in range(NMC):
                clo = csx[mc * MC]
                chi = csx[mc * MC + MC - 1]
                act = [k for k in range(NT)
                       if not (rlo[k] > chi + RADIUS + RUN_MARGIN
                               or rhi[k] < clo - RADIUS - RUN_MARGIN)]
                runs[s, mc, 0] = min(runs[s, mc, 0], act[0])
                runs[s, mc, 1] = max(runs[s, mc, 1], act[-1] + 1)
    run_list = []
    for s in range(SLOTS):
        for mc in range(NMC):
            lo, hi = int(runs[s, mc, 0]), int(runs[s, mc, 1])
            assert 0 <= lo < hi <= NT
            run_list.append((s, mc, lo, hi - lo))

    # Engine assignment per unit: greedy balance of ACT vs DVE given the
    # cost model (DVE also runs the 16 fused extraction ops).
    def act_cost(L):
        return L * MC * 0.8333 + 185.0 + 187.0

    def dve_cost(L):
        return L * MC * 1.0417 + 125.0

    stt_cost = NU * (512 * 1.0417 + 125.0)
    loads = {"act": 0.0, "dve": stt_cost}
    order = sorted(range(NU), key=lambda u: -run_list[u][3])
    conv = [""] * NU
    for u in order:
        L = run_list[u][3]
        if loads["act"] + act_cost(L) <= loads["dve"] + dve_cost(L):
            conv[u] = "act"
            loads["act"] += act_cost(L)
        else:
            conv[u] = "dve"
            loads["dve"] += dve_cost(L)

    # Emission order: ACT/DVE-interleaved (the z-PSUM pool keeps mask
    # production near emission order, so clustering either engine's units
    # would idle the other engine at the start).  SLOT_MAJOR additionally
    # groups slot 0 first so late feature DMAs overlap slot-0 compute;
    # TAIL_DVE ends with a DVE-masked unit so the final extraction follows
    # the final mask immediately.
    def interleave(units):
        acts = [u for u in units if conv[u] == "act"]
        dves = [u for u in units if conv[u] == "dve"]
        return sorted(
            units,
            key=lambda u: ((acts.index(u) + 0.5) / max(len(acts), 1)
                           if conv[u] == "act"
                           else (dves.index(u) + 0.5) / max(len(dves), 1)))

    if SLOT_MAJOR:
        emit_order = []
        for s in range(SLOTS):
            emit_order += interleave(
                [u for u in range(NU) if run_list[u][0] == s])
    else:
        emit_order = interleave(list(range(NU)))
    if TAIL_DVE:
        tail = [u for u in emit_order if conv[u] == "dve"][-1:]
        emit_order = [u for u in emit_order if u not in tail] + tail
    run_list = [run_list[u] for u in emit_order]
    conv = [conv[u] for u in emit_order]

    spec = (tuple(run_list), tuple(conv))

    # Per-core device inputs + correction data.
    in_maps = []
    combine = []
    for c in range(N_CORES):
        i, js, rho, sigs = cores[c]
        geoL, _ = _geo_rows(psc[i, rho], pdc[i, js[0], sigs[0]])
        geoR = np.zeros((SLOTS, KGEO, N), dtype=BF16)
        fj8 = np.zeros((SLOTS, NT, P, D), dtype=FP8)
        fi8 = np.zeros((SLOTS, NMC, 2, P, D), dtype=FP8)
        for s in range(SLOTS):
            _, gR = _geo_rows(psc[i, rho], pdc[i, js[s], sigs[s]])
            geoR[s] = gR
            fj8[s] = fhat8[js[s]][rho].reshape(NT, P, D)
            fi8[s] = fhat8[i][sigs[s]].reshape(NMC, 2, P, D)
        geo = np.concatenate(
            [geoL.astype(BF16)[:, None, :], geoR.transpose(1, 0, 2)], axis=1)
        in_maps.append({
            # DRAM layouts mirror the SBUF tiles exactly so every input DMA
            # is a contiguous >=512B-burst copy (full DMA bus rate).
            "fj8": np.ascontiguousarray(fj8.transpose(2, 0, 1, 3)),
            "fi8": np.ascontiguousarray(fi8.transpose(3, 0, 1, 2, 4)),
            "geo": np.ascontiguousarray(geo),
        })
        combine.append((fj8.astype(np.float64), fi8.astype(np.float64)))
    return in_maps, spec, run_list, conv, combine


# ---------------------------------------------------------------- bass build

def _build_bass(spec, reps=1):
    import concourse.bass as bass
    import concourse.mybir as mybir
    import concourse.tile as tile

    run_list, conv = spec
    nc = bass.Bass(trn_type="TRN2", target_bir_lowering=False, debug=False)
    f32 = mybir.dt.float32
    bf16 = mybir.dt.bfloat16
    fp8 = mybir.dt.float8e4

    fj_d = nc.dram_tensor("fj8", [P, SLOTS, NT, D], fp8, kind="ExternalInput")
    fi_d = nc.dram_tensor("fi8", [P, SLOTS, NMC, 2, D], fp8,
                          kind="ExternalInput")
    # geo[:, 0, :] = geoL (shared); geo[:, 1+s, :] = geoR of slot s
    geo_d = nc.dram_tensor("geo", [KGEO, 1 + SLOTS, N], bf16,
                           kind="ExternalInput")
    out_d = nc.dram_tensor("out", [P, 2 * NU], f32, kind="ExternalOutput")

    with tile.TileContext(nc) as tc:
        with (
            tc.tile_pool(name="feat", bufs=1) as feat_pool,
            tc.tile_pool(name="geo", bufs=1) as geo_pool,
            tc.tile_pool(name="acc", bufs=1) as acc_pool,
            tc.tile_pool(name="mask", bufs=4) as mask_pool,
            tc.tile_pool(name="trash", bufs=2) as trash_pool,
            tc.tile_pool(name="psum_z", bufs=3, space="PSUM") as z_pool,
            tc.tile_pool(name="psum_g", bufs=2, space="PSUM") as g_pool,
        ):
            fj_sb = feat_pool.tile([P, SLOTS, NT, D], fp8)
            fi_sb = feat_pool.tile([P, SLOTS, NMC, 2, D], fp8)
            geo_sb = geo_pool.tile([P, 1 + SLOTS, N], bf16)
            geoL_sb = geo_sb[:, 0, :]
            geoR_sb = geo_sb[:, 1:, :]
            acc = acc_pool.tile([P, 2, NU], f32)  # [ext | cnt] interleaved
            ext_acc = acc[:, 0, :]
            cnt_acc = acc[:, 1, :]

            nc.sync.dma_start(out=geo_sb[0:KGEO, :, :], in_=geo_d[:])
            for s in range(SLOTS):
                nc.sync.dma_start(out=fj_sb[:, s, :, :], in_=fj_d[:, s])
                nc.sync.dma_start(out=fi_sb[:, s, :, :, :], in_=fi_d[:, s])

            def emit_z(u):
                s, mc, lo, L = run_list[u]
                z_t = z_pool.tile([P, MAX_L * MC], f32, tag="z")
                for k in range(L):
                    nt = lo + k
                    nc.tensor.matmul(
                        z_t[:, k * MC:(k + 1) * MC],
                        geoL_sb[0:KGEO, nt * P:(nt + 1) * P],
                        geoR_sb[0:KGEO, s, mc * MC:(mc + 1) * MC],
                        start=True, stop=True)
                return z_t

            def emit_mask(u, z_t):
                s, mc, lo, L = run_list[u]
                mask_t = mask_pool.tile([P, MAX_L, MC], fp8, tag="mask")
                z_in = z_t[:, 0:L * MC].rearrange("p (l m) -> p l m", l=L)
                if conv[u] == "act":
                    nc.scalar.activation(
                        mask_t[:, 0:L, :], z_in,
                        mybir.ActivationFunctionType.Sign,
                        accum_out=cnt_acc[:, u:u + 1])
                else:
                    nc.vector.tensor_scalar(
                        out=mask_t[:, 0:L, :], in0=z_in,
                        scalar1=0.0, scalar2=0.0,
                        op0=mybir.AluOpType.is_ge, op1=mybir.AluOpType.add,
                        accum_out=cnt_acc[:, u:u + 1])
                return mask_t

            def emit_g(u, mask_t, g_t):
                s, mc, lo, L = run_list[u]
                npairs = L // 2
                for h in range(2):
                    col = h * D
                    for si in range(npairs):
                        nc.tensor.matmul(
                            g_t[:, col:col + D],
                            mask_t[:, 2 * si:2 * si + 2,
                                   h * P:(h + 1) * P],
                            fj_sb[:, s, lo + 2 * si:lo + 2 * si + 2, :],
                            start=(si == 0), stop=(si == npairs - 1 and
                                                   L % 2 == 0),
                            perf_mode=mybir.MatmulPerfMode.DoubleRow)
                    if L % 2 == 1:
                        nc.tensor.matmul(
                            g_t[:, col:col + D],
                            mask_t[:, L - 1, h * P:(h + 1) * P],
                            fj_sb[:, s, lo + L - 1, :],
                            start=(npairs == 0), stop=True)

            def emit_ext(u, g_t):
                s, mc, lo, L = run_list[u]
                tr = trash_pool.tile([P, 2 * D], fp8, tag="tr")
                nc.vector.scalar_tensor_tensor(
                    out=tr[:], in0=g_t[:, 0:2 * D], scalar=0.0,
                    in1=fi_sb[:, s, mc, :, :].rearrange("p h d -> p (h d)"),
                    op0=mybir.AluOpType.add, op1=mybir.AluOpType.mult,
                    accum_out=ext_acc[:, u:u + 1])

            def emit_body():
                stage = [None] * NU  # mask_t handoff
                for u in range(NU + 1):
                    if u < NU:
                        z_t = emit_z(u)
                        stage[u] = emit_mask(u, z_t)
                    if 1 <= u:
                        v = u - 1
                        g_t = g_pool.tile([P, 2 * D], f32, tag="g")
                        emit_g(v, stage[v], g_t)
                        emit_ext(v, g_t)
                        # Chunked result DMAs: each waits on only a few
                        # units' accum writes and overlaps later compute.
                        if v % OUT_CHUNK == OUT_CHUNK - 1:
                            lo = v - OUT_CHUNK + 1
                            nc.scalar.dma_start(
                                out=out_d[:].rearrange(
                                    "p (a u) -> p a u", a=2)[:, :, lo:v + 1],
                                in_=acc[:, :, lo:v + 1])

            if reps == 1:
                emit_body()
            else:
                with tc.For_i(0, reps, 1):
                    emit_body()

    _split_multi_waits(nc)
    return nc


def _split_multi_waits(nc):
    """Walrus rejects >1 sync-wait on compute/DMA instruction encodings.
    Hoist all but one wait onto standalone InstEventSemaphore instructions
    immediately before the instruction on the same engine queue."""
    import concourse.mybir as mybir

    n_split = 0
    for fn in nc.m.functions:
        for bb in fn.blocks:
            new_list = []
            for inst in bb.instructions:
                si = inst.sync_info
                if (
                    si is not None
                    and si.on_wait
                    and len(si.on_wait) > 1
                    and not isinstance(inst, mybir.InstEventSemaphore)
                ):
                    waits = list(si.on_wait)
                    for k, w in enumerate(waits[:-1]):
                        n_split += 1
                        new_list.append(
                            mybir.InstEventSemaphore(
                                name=f"{inst.name}-hw{k}",
                                engine=inst.engine,
                                ins=[], outs=[],
                                sync_info=mybir.SyncInfo(
                                    on_wait=[w], on_update=[]),
                            ))
                    inst.sync_info = mybir.SyncInfo(
                        on_wait=[waits[-1]],
                        on_update=list(si.on_update or []))
                new_list.append(inst)
            bb.instructions[:] = new_list
    return n_split


def _get_bass(spec, reps=1):
    key = (spec, reps)
    if key not in _CACHE:
        _CACHE[key] = _build_bass(spec, reps=reps)
    return _CACHE[key]


# ---------------------------------------------------------------- combine

def _combine(results, run_list, conv, combine):
    A_tot = 0.0
    B_tot = 0.0
    for c in range(N_CORES):
        out = results[c]["out"].astype(np.float64)
        fj64, fi64 = combine[c]
        r = out[:, 0:NU].sum(axis=0)       # ext accums
        a = out[:, NU:2 * NU].sum(axis=0)  # count accums
        for u in range(NU):
            s, mc, lo, L = run_list[u]
            if conv[u] == "act":
                area = L * P * MC
                A_tot += 0.5 * (a[u] + area)
                S = fj64[s, lo:lo + L].reshape(L * P, D).sum(axis=0)
                FS = fi64[s, mc].reshape(2 * P, D).sum(axis=0)
                C = float(np.dot(S, FS))
                B_tot += 0.5 * (r[u] + C)
            else:
                A_tot += a[u]
                B_tot += r[u]
    return A_tot, B_tot


def kernel(features, pts_src, pts_dst, invis_idx, height, width):
    global LAST
    del invis_idx  # unused by the reference computation

    features = np.asarray(features)
    pts_src = np.asarray(pts_src)
    pts_dst = np.asarray(pts_dst)

    in_maps, spec, run_list, conv, combine = _host_prep(
        features, pts_src, pts_dst, height, width)

    from concourse.bass_utils import run_bass_kernel_spmd

    nc = _get_bass(spec)
    LAST = run_bass_kernel_spmd(nc, in_maps, core_ids=list(range(N_CORES)))

    A_tot, B_tot = _combine(LAST.results, run_list, conv, combine)
    loss = (A_tot - B_tot) / max(A_tot, 1.0)
    return np.float32(loss)
